# revision 1
# baseline (speedup 1.0000x reference)
"""Trainium2 (8 NeuronCores) Bass kernel for the DynamicGNN problem.

Self-contained: plans the graph partition on the host (pure integer index
manipulation), builds a static SPMD Bass program at runtime, compiles and
runs it via run_bass_kernel_spmd, and returns the full [64, 2] output.

Math (identical to the reference after refactoring):
  deg[d]  = (#in-edges of d) + 1 ; dinv = rsqrt(deg)      (computed on device)
  hs      = (x @ W) * dinv[:, None]
  agg[d]  = dinv[d] * ( sum_{e: dst=d} hs[src_e] + hs[d] )
  x'      = relu(LN(agg + b) * g + be)
  gate    = relu(x2 @ Wg1 + bg1) @ Wg2 + bg2 ; e = exp(gate)  (softmax shift
            cancels algebraically; gate range is O(1) so exp is safe)
  pooled  = segsum(e * x2) / segsum(e)     (one AllReduce across cores)
  out     = relu(pooled @ Wc1 + bc1) @ Wc2 + bc2

Distribution: nodes are relabelled and packed into 8 cores x BINS bins of
<= 32 dst-nodes with <= 256 lo-side and <= 256 hi-side in-edges ("lo/hi" =
source in the lower/upper half of the relabelled id space so gather indices
fit dma_gather's int16). Each bin = 4 edge-tiles of 128 edges (2 lo + 2 hi).
The hs table is AllGathered in bf16; messages are fetched with dma_gather
and segment-summed with PE matmuls against per-tile one-hot matrices
([128 tokens, 32 dsts], built once on device and kept SBUF-resident).
"""

import numpy as np
from contextlib import ExitStack

import concourse.bacc as bacc
import concourse.tile as tile
from concourse import bass, mybir, bass_utils
from concourse.masks import make_identity

N_CORES = 8
G = 64
F_IN, H, PH = 128, 256, 128
EPS = 1e-5

BIN_NODES = 32
SIDE_CAP = 256  # lo/hi edge capacity per bin (2 tiles of 128 each)
PAD_SLOT = 99.0
NB = 3  # bins per gather chunk
AGG_BINS = None  # debug: limit #bins processed in aggregate
AGG_SKIP_MM = False  # debug: skip matmuls/evict/LN (gathers only)
SINGLE_PACKET = True

f32, bf16, i16, i32 = (
    mybir.dt.float32,
    mybir.dt.bfloat16,
    mybir.dt.int16,
    mybir.dt.int32,
)
AF = mybir.ActivationFunctionType
OP = mybir.AluOpType

# ----------------------------------------------------------------------------
# host-side planner
# ----------------------------------------------------------------------------


def _assign_cores(cnt):
    n_nodes = cnt.shape[0]
    order = np.argsort(-cnt, kind="stable")
    node_core = np.empty(n_nodes, dtype=np.int32)
    core_load = np.zeros(N_CORES, dtype=np.int64)
    core_n = np.zeros(N_CORES, dtype=np.int64)
    node_cap = -(-n_nodes // N_CORES)
    big = np.iinfo(np.int64).max
    for n in order:
        c = int(np.argmin(np.where(core_n < node_cap, core_load, big)))
        node_core[n] = c
        core_load[c] += cnt[n]
        core_n[c] += 1
    return node_core


def _pack_bins(nodes, lo_d, hi_d, bins):
    o = np.argsort(-(lo_d + hi_d), kind="stable")
    lo_room = np.full(bins, SIDE_CAP, dtype=np.int64)
    hi_room = np.full(bins, SIDE_CAP, dtype=np.int64)
    n_room = np.full(bins, BIN_NODES, dtype=np.int64)
    bin_of = np.empty(len(nodes), dtype=np.int64)
    slot_of = np.empty(len(nodes), dtype=np.int64)
    for j in o:
        ld, hd = lo_d[j], hi_d[j]
        feas = (n_room > 0) & (lo_room >= ld) & (hi_room >= hd)
        if not feas.any():
            return None
        score = np.where(feas, np.minimum(lo_room - ld, hi_room - hd), -1)
        w = int(np.argmax(score))
        bin_of[j] = w
        slot_of[j] = BIN_NODES - n_room[w]
        lo_room[w] -= ld
        hi_room[w] -= hd
        n_room[w] -= 1
    return bin_of, slot_of


def plan(edge_index, batch, n_nodes):
    src = np.asarray(edge_index[0], dtype=np.int64)
    dst = np.asarray(edge_index[1], dtype=np.int64)
    batch = np.asarray(batch, dtype=np.int64)

    cnt = np.bincount(dst, minlength=n_nodes).astype(np.int64)
    node_core = _assign_cores(cnt)

    src_is_lo = node_core[src] < 4
    lo_cnt = np.bincount(dst[src_is_lo], minlength=n_nodes).astype(np.int64)
    hi_cnt = cnt - lo_cnt

    need = 0
    for c in range(N_CORES):
        m = node_core == c
        need = max(
            need,
            -(-int(m.sum()) // BIN_NODES),
            -(-int(lo_cnt[m].sum()) // SIDE_CAP),
            -(-int(hi_cnt[m].sum()) // SIDE_CAP),
        )
    bins = -(-need // 4) * 4

    node_newid = np.empty(n_nodes, dtype=np.int64)
    for _attempt in range(4):
        npc = bins * BIN_NODES
        ok = True
        for c in range(N_CORES):
            nodes_c = np.where(node_core == c)[0]
            r = _pack_bins(nodes_c, lo_cnt[nodes_c], hi_cnt[nodes_c], bins)
            if r is None:
                ok = False
                break
            bin_of, slot_of = r
            node_newid[nodes_c] = c * npc + bin_of * BIN_NODES + slot_of
        if ok:
            break
        bins += 4
    else:
        raise RuntimeError("bin packing failed")

    npc = bins * BIN_NODES
    R = N_CORES * npc
    HALF = R // 2
    assert HALF < 32768 and npc % 128 == 0

    new_src = node_newid[src]
    new_dst = node_newid[dst]
    e_core = new_dst // npc
    e_bin = (new_dst % npc) // BIN_NODES
    e_slot = new_dst % BIN_NODES
    e_side = (new_src >= HALF).astype(np.int64)

    key = (e_core * bins + e_bin) * 2 + e_side
    order = np.argsort(key, kind="stable")
    ks, ss, sl = key[order], new_src[order], e_slot[order]
    n_groups = N_CORES * bins * 2
    g_start = np.searchsorted(ks, np.arange(n_groups), side="left")
    g_end = np.searchsorted(ks, np.arange(n_groups), side="right")

    flat_idx = np.zeros((N_CORES, 2, bins * 2 * 128), dtype=np.int16)
    slot_pm = np.full((N_CORES, 128, bins * 4), PAD_SLOT, dtype=np.float32)
    for g in range(n_groups):
        a, b = g_start[g], g_end[g]
        k = b - a
        if k == 0:
            continue
        side = g % 2
        w = (g // 2) % bins
        c = g // (2 * bins)
        assert k <= SIDE_CAP
        base = ss[a:b] - (HALF if side else 0)
        flat_idx[c, side, w * 2 * 128 + np.arange(k)] = base.astype(np.int16)
        t_loc = np.arange(k) // 128
        lane = np.arange(k) % 128
        slot_pm[c, lane, w * 4 + side * 2 + t_loc] = sl[a:b]

    ntok = bins * 2 * 128
    idx_arrs = np.zeros((N_CORES, 2, 128, ntok // 16), dtype=np.int16)
    i_all = np.arange(ntok)
    for c in range(N_CORES):
        for side in range(2):
            a16 = np.zeros((16, ntok // 16), dtype=np.int16)
            a16[i_all % 16, i_all // 16] = flat_idx[c, side]
            idx_arrs[c, side] = np.tile(a16, (8, 1))

    inv = np.full(R, -1, dtype=np.int64)
    inv[node_newid] = np.arange(n_nodes)
    valid = inv >= 0
    bids = np.full(R, -1.0, dtype=np.float32)
    bids[valid] = batch[inv[valid]].astype(np.float32)

    return dict(
        bins=bins, npc=npc, R=R, HALF=HALF, inv=inv,
        idx_arrs=idx_arrs, slot_pm=slot_pm, bids=bids,
    )


# ----------------------------------------------------------------------------
# device program
# ----------------------------------------------------------------------------


def _bcast(dram_ap, parts, free):
    return bass.AP(
        tensor=dram_ap.tensor, offset=dram_ap.offset, ap=[[0, parts], [1, free]]
    )


def build(bins, stop_stage=99, reps=1):
    nc = bacc.Bacc("TRN2", target_bir_lowering=False, debug=False, num_devices=N_CORES, num_swdge_queues=2)

    _build_body(nc, bins, stop_stage, reps)
    nc.compile()
    return nc


def _build_body(nc, bins, stop_stage, reps):
    npc = bins * BIN_NODES
    R = N_CORES * npc
    HALF = R // 2
    nblk = npc // 128
    ncols = bins * 2 * 8
    chunks = [(w0, min(w0 + NB, bins)) for w0 in range(0, bins, NB)]
    grp = list(range(N_CORES))
    xT_in = nc.dram_tensor("xT", [F_IN, npc], f32, kind="ExternalInput").ap()
    idxlo_in = nc.dram_tensor("idxlo", [128, ncols], i16, kind="ExternalInput").ap()
    idxhi_in = nc.dram_tensor("idxhi", [128, ncols], i16, kind="ExternalInput").ap()
    slot_in = nc.dram_tensor("dstslot", [128, bins * 4], f32, kind="ExternalInput").ap()
    bids_in = nc.dram_tensor("bids", [128, nblk], f32, kind="ExternalInput").ap()
    W1_in = nc.dram_tensor("W1", [F_IN, H], f32, kind="ExternalInput").ap()
    W2_in = nc.dram_tensor("W2", [H, H], f32, kind="ExternalInput").ap()
    Wg1_in = nc.dram_tensor("Wg1", [H, PH], f32, kind="ExternalInput").ap()
    Wg2r_in = nc.dram_tensor("Wg2r", [1, PH], f32, kind="ExternalInput").ap()
    Wc1_in = nc.dram_tensor("Wc1", [H, PH], f32, kind="ExternalInput").ap()
    Wc2_in = nc.dram_tensor("Wc2", [PH, 2], f32, kind="ExternalInput").ap()
    vec_ins = {}
    for nm, d in [
        ("b1", H), ("g1", H), ("be1", H), ("b2", H), ("g2", H), ("be2", H),
        ("bg1", PH), ("bg2", 1), ("bc1", PH), ("bc2", 2),
    ]:
        vec_ins[nm] = nc.dram_tensor(nm, [1, d], f32, kind="ExternalInput").ap()
    out_d = nc.dram_tensor("out", [G, 2], f32, kind="ExternalOutput").ap()

    with tile.TileContext(nc) as tc, ExitStack() as ctx:
        dram = ctx.enter_context(tc.tile_pool(name="dram", bufs=1, space="DRAM"))
        bigp = ctx.enter_context(tc.tile_pool(name="bigp", bufs=1))
        cst = ctx.enter_context(tc.tile_pool(name="cst", bufs=1))
        stg = ctx.enter_context(tc.tile_pool(name="stg", bufs=2))
        tokp = ctx.enter_context(tc.tile_pool(name="tokp", bufs=6))
        accp = ctx.enter_context(tc.tile_pool(name="accp", bufs=4))
        evst = ctx.enter_context(tc.tile_pool(name="evst", bufs=4))
        smlp = ctx.enter_context(tc.tile_pool(name="smlp", bufs=4))
        hslp = ctx.enter_context(tc.tile_pool(name="hslp", bufs=2))
        ps_win = ctx.enter_context(tc.tile_pool(name="ps_win", bufs=4, space="PSUM"))
        ps_msc = ctx.enter_context(tc.tile_pool(name="ps_msc", bufs=2, space="PSUM"))
        ps_pool = ctx.enter_context(tc.tile_pool(name="ps_pool", bufs=1, space="PSUM"))

        def one_rep():
            # ================= constants & inputs =================
            iota_i = cst.tile([128, 64], i32)
            nc.gpsimd.iota(iota_i[:], pattern=[[1, 64]], base=0, channel_multiplier=0)
            iota64 = cst.tile([128, 64], f32)
            nc.vector.tensor_copy(out=iota64[:], in_=iota_i[:])

            ident_bf = cst.tile([128, 128], bf16)
            make_identity(nc, ident_bf[:])
            ident_f = cst.tile([128, 128], f32)
            make_identity(nc, ident_f[:])

            ones_bf = cst.tile([128, 1], bf16)
            nc.vector.memset(ones_bf[:], 1.0)
            eps_t = cst.tile([128, 1], f32)
            nc.vector.memset(eps_t[:], EPS)

            idxlo_sb = bigp.tile([128, ncols], i16)
            nc.sync.dma_start(out=idxlo_sb[:], in_=idxlo_in[:])
            idxhi_sb = bigp.tile([128, ncols], i16)
            nc.sync.dma_start(out=idxhi_sb[:], in_=idxhi_in[:])
            slot_sb = bigp.tile([128, bins * 4], f32)
            nc.sync.dma_start(out=slot_sb[:], in_=slot_in[:])
            bids_sb = cst.tile([128, nblk], f32)
            nc.sync.dma_start(out=bids_sb[:], in_=bids_in[:])

            # weights
            def load_cast_bf(src_ap, shape, name):
                t0 = stg.tile(shape, f32, tag="wstg", name=f"{name}_stg")
                nc.sync.dma_start(out=t0[:], in_=src_ap)
                t1 = cst.tile(shape, bf16, name=name)
                nc.vector.tensor_copy(out=t1[:], in_=t0[:])
                return t1

            W1_bf = load_cast_bf(W1_in[:], [128, H], "W1bf")
            W2_bf = load_cast_bf(
                W2_in[:].rearrange("(a p) h -> p a h", a=2), [128, 2, H], "W2bf"
            )
            Wg1_bf = load_cast_bf(
                Wg1_in[:].rearrange("(a p) h -> p a h", a=2), [128, 2, PH], "Wg1bf"
            )
            Wc1_f = cst.tile([128, 2, PH], f32)
            nc.sync.dma_start(out=Wc1_f[:], in_=Wc1_in[:].rearrange("(a p) h -> p a h", a=2))
            Wc2_f = cst.tile([128, 2], f32)
            nc.sync.dma_start(out=Wc2_f[:], in_=Wc2_in[:])

            bcv = {}
            for nm, d, parts in [
                ("b1", H, 128), ("g1", H, 128), ("be1", H, 128),
                ("b2", H, 128), ("g2", H, 128), ("be2", H, 128),
                ("bg1", PH, 128), ("bg2", 1, 128),
                ("bc1", PH, 64), ("bc2", 2, 64),
            ]:
                t = cst.tile([parts, d], f32, name=f"{nm}_bc")
                nc.sync.dma_start(out=t[:], in_=_bcast(vec_ins[nm][:], parts, d))
                bcv[nm] = t
            bcv_bf = {}
            for nm in ["b1", "g1", "be1", "b2", "g2", "be2"]:
                tbf = cst.tile([128, H], bf16, name=f"{nm}_bf")
                nc.vector.tensor_copy(out=tbf[:], in_=bcv[nm][:])
                bcv_bf[nm] = tbf
            wg2_bc = cst.tile([128, PH], f32)
            nc.sync.dma_start(out=wg2_bc[:], in_=_bcast(Wg2r_in[:], 128, PH))

            # xT load + cast (chunked staging)
            xT_bf = bigp.tile([128, 2, npc], bf16, tag="xtbuf", name="xT_bf")
            for k in range(0, npc, 512):
                kk = min(512, npc - k)
                t0 = stg.tile([128, 512], f32, tag="xstg", name="x_stg")
                nc.sync.dma_start(out=t0[:, :kk], in_=xT_in[:, k : k + kk])
                nc.vector.tensor_copy(out=xT_bf[:, 0, k : k + kk], in_=t0[:, :kk])

            # ====== seg matrices (built once) + deg counts, interleaved ======
            seg = bigp.tile([128, bins * 4, 32], bf16, tag="segbuf", name="seg")
            deg_sb = cst.tile([128, nblk], f32)
            for w in range(bins):
                in0 = bass.AP(
                    tensor=iota64.tensor,
                    offset=iota64[:].offset,
                    ap=[iota64[:].ap[0], [0, 4], [1, 32]],
                )
                sslice = slot_sb[:, 4 * w : 4 * w + 4]
                in1 = bass.AP(
                    tensor=slot_sb.tensor,
                    offset=sslice.offset,
                    ap=[sslice.ap[0], sslice.ap[1], [0, 32]],
                )
                nc.vector.tensor_tensor(
                    out=seg[:, 4 * w : 4 * w + 4, :], in0=in0, in1=in1, op=OP.is_equal
                )
                cps = ps_win.tile([32, H], f32, tag="wps", name="cnt_ps")
                for t in range(4):
                    nc.tensor.matmul(
                        out=cps[:, 0:1],
                        lhsT=seg[:, 4 * w + t, :],
                        rhs=ones_bf[:],
                        start=(t == 0),
                        stop=(t == 3),
                    )
                j = w % 4
                nc.vector.tensor_scalar(
                    out=deg_sb[32 * j : 32 * j + 32, w // 4 : w // 4 + 1],
                    in0=cps[:, 0:1],
                    scalar1=1.0,
                    scalar2=None,
                    op0=OP.add,
                )
            sqd = cst.tile([128, nblk], f32)
            nc.scalar.activation(out=sqd[:], in_=deg_sb[:], func=AF.Sqrt)
            dinv = cst.tile([128, nblk], f32)
            nc.vector.reciprocal(out=dinv[:], in_=sqd[:])

            # ================= shared DRAM =================
            hs_dram = [
                dram.tile([npc, H], bf16, name="hs_dram1"),
                dram.tile([npc, H], bf16, name="hs_dram2"),
            ]
            tables = [
                dram.tile([R, H], bf16, addr_space="Shared", name="table1"),
                dram.tile([R, H], bf16, addr_space="Shared", name="table2"),
            ]

            def dense_hs(layer, lhsT_halves, rhs_halves):
                for nb in range(nblk):
                    hps = ps_msc.tile([128, H], f32, tag="mps", name="h_ps")
                    nh = len(rhs_halves)
                    for hh in range(nh):
                        nc.tensor.matmul(
                            out=hps[:],
                            lhsT=lhsT_halves[hh](nb),
                            rhs=rhs_halves[hh],
                            start=(hh == 0),
                            stop=(hh == nh - 1),
                        )
                    nc.scalar.activation(
                        out=hs_sb[:, nb, :], in_=hps[:], func=AF.Copy,
                        scale=dinv[:, nb : nb + 1],
                    )
                    nc.sync.dma_start(
                        out=hs_dram[layer][nb * 128 : (nb + 1) * 128, :],
                        in_=hs_sb[:, nb, :],
                    )

            def aggregate(layer, g_v, be_v, b_v, epilogue=None):
                # g_v/be_v/b_v are bf16 broadcast tiles
                nbins_do = AGG_BINS if AGG_BINS is not None else bins
                table = tables[layer]
                tok_lo = tok_hi = None
                ch_iter = iter(chunks)
                cur_end = 0
                w_base = 0
                a_g = None
                cur_hsl = None
                for w in range(nbins_do):
                    if w >= cur_end:
                        w0, w1 = next(ch_iter)
                        cur_end = w1
                        w_base = w0
                        nt = (w1 - w0) * 2
                        tok_lo = tokp.tile([128, 2 * NB, H], bf16, tag="tok", name="tok_lo")
                        tok_hi = tokp.tile([128, 2 * NB, H], bf16, tag="tok", name="tok_hi")
                        nc.gpsimd.dma_gather(
                            out_ap=tok_lo[:, :nt, :],
                            in_ap=table[0:HALF, :],
                            idxs_ap=idxlo_sb[:, w0 * 16 : w0 * 16 + nt * 8],
                            num_idxs=nt * 128,
                            num_idxs_reg=nt * 128,
                            elem_size=H,
                            single_packet=SINGLE_PACKET,
                        )
                        nc.gpsimd.dma_gather(
                            out_ap=tok_hi[:, :nt, :],
                            in_ap=table[HALF:R, :],
                            idxs_ap=idxhi_sb[:, w0 * 16 : w0 * 16 + nt * 8],
                            num_idxs=nt * 128,
                            num_idxs_reg=nt * 128,
                            elem_size=H,
                            single_packet=SINGLE_PACKET,
                        )
                    if AGG_SKIP_MM:
                        continue
                    g = w // 4
                    j = w % 4
                    if j == 0:
                        a_g = accp.tile([128, H], bf16, tag="agg", name="a_g")
                    wps = ps_win.tile([32, H], f32, tag="wps", name="w_ps")
                    side_rhs = [
                        tok_lo[:, (w - w_base) * 2 + 0, :],
                        tok_lo[:, (w - w_base) * 2 + 1, :],
                        tok_hi[:, (w - w_base) * 2 + 0, :],
                        tok_hi[:, (w - w_base) * 2 + 1, :],
                    ]
                    for t in range(4):
                        nc.tensor.matmul(
                            out=wps[:],
                            lhsT=seg[:, 4 * w + t, :],
                            rhs=side_rhs[t],
                            start=(t == 0),
                            stop=(t == 3),
                        )
                    nc.vector.tensor_tensor(
                        out=a_g[32 * j : 32 * j + 32, :],
                        in0=wps[:],
                        in1=hs_sb[32 * j : 32 * j + 32, g, :],
                        op=OP.add,
                    )
                    if j == 3:
                        nc.vector.tensor_scalar(
                            out=a_g[:], in0=a_g[:], scalar1=dinv[:, g : g + 1],
                            scalar2=None, op0=OP.mult,
                        )
                        nc.vector.tensor_tensor(
                            out=a_g[:], in0=a_g[:], in1=b_v[:], op=OP.add
                        )
                        st = smlp.tile([128, 6], f32, tag="bnst", name="bn_st")
                        nc.vector.bn_stats(out=st[:], in_=a_g[:])
                        mv = smlp.tile([128, 2], f32, tag="bnmv", name="bn_mv")
                        nc.vector.bn_aggr(out=mv[:], in_=st[:])
                        sd = smlp.tile([128, 1], f32, tag="sd", name="sd_t")
                        nc.scalar.activation(
                            out=sd[:], in_=mv[:, 1:2], func=AF.Sqrt, bias=eps_t[:]
                        )
                        rstd = smlp.tile([128, 1], f32, tag="rstd", name="rstd_t")
                        nc.vector.reciprocal(out=rstd[:], in_=sd[:])
                        nmb = smlp.tile([128, 1], f32, tag="nmb", name="nmb_t")
                        nc.vector.tensor_tensor(
                            out=nmb[:], in0=mv[:, 0:1], in1=rstd[:], op=OP.mult
                        )
                        nc.vector.tensor_scalar(
                            out=nmb[:], in0=nmb[:], scalar1=-1.0, scalar2=None, op0=OP.mult
                        )
                        nc.scalar.activation(
                            out=a_g[:], in_=a_g[:], func=AF.Identity,
                            bias=nmb[:], scale=rstd[:],
                        )
                        nc.vector.tensor_tensor(
                            out=a_g[:], in0=a_g[:], in1=g_v[:], op=OP.mult
                        )
                        nc.vector.tensor_tensor(
                            out=a_g[:], in0=a_g[:], in1=be_v[:], op=OP.add
                        )
                        nc.scalar.activation(out=x_sb[:, g, :], in_=a_g[:], func=AF.Relu)
                        if epilogue is not None:
                            epilogue(g)

            def transpose_x(dst):
                for nb in range(nblk):
                    for hh in range(2):
                        tps = ps_msc.tile([128, 128], bf16, tag="mps", name="t_ps")
                        nc.tensor.transpose(
                            out=tps[:],
                            in_=x_sb[:, nb, hh * 128 : hh * 128 + 128],
                            identity=ident_bf[:],
                        )
                        nc.vector.tensor_copy(
                            out=dst[:, hh, nb * 128 : (nb + 1) * 128], in_=tps[:]
                        )

            x_sb = bigp.tile([128, nblk, H], bf16, name="x_sb")
            hs_sb = bigp.tile([128, nblk, H], bf16, name="hs_sb")

            def l1_epilogue(g):
                for hh in range(2):
                    tps = ps_msc.tile([128, 128], bf16, tag="mps", name="t_ps")
                    nc.tensor.transpose(
                        out=tps[:],
                        in_=x_sb[:, g, hh * 128 : hh * 128 + 128],
                        identity=ident_bf[:],
                    )
                    nc.vector.tensor_copy(
                        out=xT_bf[:, hh, g * 128 : (g + 1) * 128], in_=tps[:]
                    )

            # ================= layer 1 =================
            def early_out(tag_v):
                nc.vector.memset(osb_e[:], tag_v)
                nc.sync.dma_start(out=out_d[:], in_=osb_e[:])

            osb_e = cst.tile([64, 2], f32, name="osb_e")
            if stop_stage <= 2:
                early_out(float(stop_stage))
                return
            dense_hs(0, [lambda nb: xT_bf[:, 0, nb * 128 : (nb + 1) * 128]], [W1_bf[:]])
            if stop_stage <= 3:
                early_out(3.0)
                return
            nc.gpsimd.collective_compute(
                "AllGather", OP.bypass, replica_groups=[grp],
                ins=[hs_dram[0][:].opt()], outs=[tables[0][:].opt()],
            )
            if stop_stage <= 4:
                early_out(4.0)
                return
            aggregate(0, bcv_bf["g1"], bcv_bf["be1"], bcv_bf["b1"], epilogue=l1_epilogue)
            if stop_stage <= 5:
                early_out(5.0)
                return

            # ================= layer 2 =================
            dense_hs(
                1,
                [
                    lambda nb: xT_bf[:, 0, nb * 128 : (nb + 1) * 128],
                    lambda nb: xT_bf[:, 1, nb * 128 : (nb + 1) * 128],
                ],
                [W2_bf[:, 0, :], W2_bf[:, 1, :]],
            )
            nc.gpsimd.collective_compute(
                "AllGather", OP.bypass, replica_groups=[grp],
                ins=[hs_dram[1][:].opt()], outs=[tables[1][:].opt()],
            )
            if stop_stage <= 6:
                early_out(6.0)
                return

            # ====== pooling: fused into layer-2 aggregation as per-group epilogue ======
            pool_ps = ps_pool.tile([64, H], f32, tag="pps", name="pool_ps")
            den_ps = ps_pool.tile([64, 1], f32, tag="dps", name="den_ps")
            gate_col_sb = cst.tile([128, nblk], f32, name="gate_col_sb")

            def pool_epilogue(g):
                # x2T for this block (feeds the gate matmuls)
                for hh in range(2):
                    tps = ps_msc.tile([128, 128], bf16, tag="mps", name="t_ps")
                    nc.tensor.transpose(
                        out=tps[:],
                        in_=x_sb[:, g, hh * 128 : hh * 128 + 128],
                        identity=ident_bf[:],
                    )
                    nc.vector.tensor_copy(
                        out=xT_bf[:, hh, g * 128 : (g + 1) * 128], in_=tps[:]
                    )
                gps = ps_msc.tile([128, PH], f32, tag="mps", name="g_ps")
                for hh in range(2):
                    nc.tensor.matmul(
                        out=gps[:],
                        lhsT=xT_bf[:, hh, g * 128 : (g + 1) * 128],
                        rhs=Wg1_bf[:, hh, :],
                        start=(hh == 0),
                        stop=(hh == 1),
                    )
                gt = smlp.tile([128, PH], f32, tag="gt", name="g_t")
                nc.vector.tensor_tensor(
                    out=gt[:], in0=gps[:], in1=bcv["bg1"][:], op=OP.add
                )
                gr = smlp.tile([128, PH], bf16, tag="gr", name="g_r")
                nc.scalar.activation(out=gr[:], in_=gt[:], func=AF.Relu)
                g2t = smlp.tile([128, PH], f32, tag="g2t", name="g2_t")
                nc.vector.tensor_tensor(
                    out=g2t[:], in0=gr[:], in1=wg2_bc[:], op=OP.mult
                )
                nc.vector.tensor_reduce(
                    out=gate_col_sb[:, g : g + 1], in_=g2t[:],
                    axis=mybir.AxisListType.X, op=OP.add,
                )

            aggregate(1, bcv_bf["g2"], bcv_bf["be2"], bcv_bf["b2"], epilogue=pool_epilogue)

            # pooling tail: one Exp batch, then one-hot*e matmuls
            e_sb = cst.tile([128, nblk], f32, name="e_sb")
            nc.scalar.activation(
                out=e_sb[:], in_=gate_col_sb[:], func=AF.Exp, bias=bcv["bg2"][:, 0:1]
            )
            for g in range(nblk):
                B_e = smlp.tile([128, G], bf16, tag="be", name="B_e")
                nc.vector.tensor_scalar(
                    out=B_e[:], in0=iota64[:], scalar1=bids_sb[:, g : g + 1],
                    scalar2=e_sb[:, g : g + 1], op0=OP.is_equal, op1=OP.mult,
                )
                nc.tensor.matmul(
                    out=pool_ps[:], lhsT=B_e[:], rhs=x_sb[:, g, :],
                    start=(g == 0), stop=(g == nblk - 1),
                )
                nc.tensor.matmul(
                    out=den_ps[:], lhsT=B_e[:], rhs=ones_bf[:],
                    start=(g == 0), stop=(g == nblk - 1),
                )

            ar_sb = cst.tile([64, H + 1], f32, name="ar_sb")
            nc.vector.tensor_copy(out=ar_sb[:, :H], in_=pool_ps[:])
            nc.vector.tensor_copy(out=ar_sb[:, H : H + 1], in_=den_ps[:])
            ar_in = dram.tile([64, H + 1], f32, name="ar_in")
            nc.sync.dma_start(out=ar_in[:], in_=ar_sb[:])
            ar_out = dram.tile([64, H + 1], f32, addr_space="Shared", name="ar_out")
            nc.gpsimd.collective_compute(
                "AllReduce", OP.add, replica_groups=[grp],
                ins=[ar_in[:].opt()], outs=[ar_out[:].opt()],
            )
            arr = cst.tile([64, H + 1], f32, name="arr")
            nc.sync.dma_start(out=arr[:], in_=ar_out[:])

            recip = cst.tile([64, 1], f32, name="recip")
            nc.vector.reciprocal(out=recip[:], in_=arr[:, H : H + 1])
            pooled = cst.tile([64, H], f32, name="pooled")
            nc.vector.tensor_scalar(
                out=pooled[:], in0=arr[:, :H], scalar1=recip[:],
                scalar2=None, op0=OP.mult,
            )
            pooledT = cst.tile([128, 2, 64], f32, name="pooledT")
            for hh in range(2):
                tpsf = ps_msc.tile([128, 64], f32, tag="mps", name="tf_ps")
                nc.tensor.transpose(
                    out=tpsf[:],
                    in_=pooled[:, hh * 128 : hh * 128 + 128],
                    identity=ident_f[:64, :64],
                )
                nc.vector.tensor_copy(out=pooledT[:, hh, :], in_=tpsf[:])
            c1ps = ps_msc.tile([64, PH], f32, tag="mps", name="c1_ps")
            for hh in range(2):
                nc.tensor.matmul(
                    out=c1ps[:], lhsT=pooledT[:, hh, :], rhs=Wc1_f[:, hh, :],
                    start=(hh == 0), stop=(hh == 1),
                )
            c1 = cst.tile([64, PH], f32, name="c1")
            nc.vector.tensor_tensor(out=c1[:], in0=c1ps[:], in1=bcv["bc1"][:], op=OP.add)
            c1r = cst.tile([64, PH], f32, name="c1r")
            nc.scalar.activation(out=c1r[:], in_=c1[:], func=AF.Relu)
            c1T = cst.tile([128, 64], f32, name="c1T")
            tpsf2 = ps_msc.tile([128, 64], f32, tag="mps", name="tf2_ps")
            nc.tensor.transpose(out=tpsf2[:], in_=c1r[:], identity=ident_f[:64, :64])
            nc.vector.tensor_copy(out=c1T[:], in_=tpsf2[:])
            ops = ps_msc.tile([64, 2], f32, tag="mps", name="o_ps")
            nc.tensor.matmul(out=ops[:], lhsT=c1T[:], rhs=Wc2_f[:], start=True, stop=True)
            osb = cst.tile([64, 2], f32, name="osb")
            nc.vector.tensor_tensor(out=osb[:], in0=ops[:], in1=bcv["bc2"][:], op=OP.add)
            nc.sync.dma_start(out=out_d[:], in_=osb[:])

        for _rep in range(reps):
            one_rep()


# ----------------------------------------------------------------------------
# entry point
# ----------------------------------------------------------------------------

_CACHE = {}


def make_in_maps(inputs, pl):
    x = np.asarray(inputs["x"], np.float32)
    bins, npc, R = pl["bins"], pl["npc"], pl["R"]
    inv = pl["inv"]
    valid = inv >= 0
    xr = np.zeros((R, F_IN), np.float32)
    xr[valid] = x[inv[valid]]
    bids_all = pl["bids"].reshape(N_CORES, npc)
    w_np = {
        k: np.asarray(inputs[k], np.float32)
        for k in [
            "W1", "W2", "Wg1", "Wg2", "Wc1", "Wc2",
            "b1", "g1", "be1", "b2", "g2", "be2", "bg1", "bg2", "bc1", "bc2",
        ]
    }
    in_maps = []
    nloc = np.arange(npc)
    for c in range(N_CORES):
        bid_pm = np.full((128, npc // 128), -1.0, np.float32)
        bid_pm[nloc % 128, nloc // 128] = bids_all[c]
        m = {
            "xT": np.ascontiguousarray(xr[c * npc : (c + 1) * npc].T),
            "idxlo": pl["idx_arrs"][c, 0],
            "idxhi": pl["idx_arrs"][c, 1],
            "dstslot": pl["slot_pm"][c],
            "bids": bid_pm,
            "W1": w_np["W1"],
            "W2": w_np["W2"],
            "Wg1": w_np["Wg1"],
            "Wg2r": np.ascontiguousarray(w_np["Wg2"].reshape(PH, 1).T),
            "Wc1": w_np["Wc1"],
            "Wc2": w_np["Wc2"],
        }
        for nm in ["b1", "g1", "be1", "b2", "g2", "be2", "bg1", "bg2", "bc1", "bc2"]:
            m[nm] = np.ascontiguousarray(w_np[nm].reshape(1, -1))
        in_maps.append(m)
    return in_maps


def kernel(**inputs):
    x = np.asarray(inputs["x"], np.float32)
    pl = plan(
        np.asarray(inputs["edge_index"]), np.asarray(inputs["batch"]), x.shape[0]
    )
    key = (pl["bins"],)
    if key not in _CACHE:
        _CACHE[key] = build(pl["bins"])
    nc = _CACHE[key]
    in_maps = make_in_maps(inputs, pl)
    res = bass_utils.run_bass_kernel_spmd(nc, in_maps, core_ids=list(range(N_CORES)))
    return np.asarray(res.results[0]["out"], np.float32)

